# revision 1
# baseline (speedup 1.0000x reference)
"""BatchAllTripletLoss kernel for 8 Trainium2 NeuronCores.

Reference computation:
    pd = pairwise_euclidean(rep)                        # [512, 512]
    tl[a,p,k] = relu(pd[a,p] - pd[a,k] + 5.0) * mask    # [512, 512, 512]
    loss = sum(tl) / (count(tl > eps) + eps)

The mask (p!=a, k!=a, p!=k, label[p]==label[a], label[k]!=label[a])
collapses: label[p]==label[a] and label[k]!=label[a] imply p!=k and k!=a,
so valid triplets are exactly (anchor-positive pairs) x (k with a
different label).  With 64 labels over 512 rows there are only ~4100
(a,p) pairs, so instead of a dense [N,N,N] sweep each core processes its
anchors' pairs as rows of [128-pair, 512-k] tiles:

  per core (64 anchors):
    d[64,512]   = sqrt(relu(aug-matmul))            PE + DVE + ACT
    ym          = d + BIGM*same_label               DVE
    per pair-tile t:
      Gym       = sel_t.T @ ym                      PE one-hot row gather
      x[p]      = sum_k (iota==pidx)*Gym            DVE; = d[a,p] + BIGM
      xp        = x + (margin - BIGM)               DVE
      S_t[p]    = sum_k relu(xp - Gym)              ACT accum
      C_t[p]    = sum_k (Gym < xp)                  DVE accum
    out[1,2*Tp] = ones.T @ [S | C]                  PE partition sum

All matmuls run in float32r (single-pass fp32, ~2^-13 relative rounding;
the one-hot gather then carries that rounding into d).  BIGM = 128 both
masks out same-label k columns (xp <= ~35 << 128 so relu/count give
exactly 0) and carries the bias through the gather; the combined
rounding is ~1e-2 absolute per term, mean-zero, ~1e-4 on the final sums.
rep arrives both row-major (for the row-norm accumulates) and
host-transposed (pure layout permutation) so no PE transposes are
needed.  Anchors are block-sharded 64 per core; the 8 partial
(sum, count) pairs are reduced on the host (the all-reduce of the
sharding hint).  Host-side prep is integer/mask/layout logic only; all
float arithmetic runs on device.
"""

import ml_dtypes
import numpy as np

import concourse.bass as bass
import concourse.tile as tile
from concourse import bacc, mybir
from concourse.bass_utils import run_bass_kernel_spmd
from concourse.vector_clock import ScopedClock


_orig_aeb = bass.Bass.all_engine_barrier


def _skip_const_barrier(self, *, sem_only=False):
    if not getattr(self, "_aeb_skipped_once", False):
        self._aeb_skipped_once = True
        return
    return _orig_aeb(self, sem_only=sem_only)


def _cheap_drain_and_barrier(self, tick_clock, wait_clock):
    """Exit protocol with sequencer-only barriers: the SP drain already
    waits out every engine/DMA tick of the tile clock, so the per-engine
    pipeline drains of the stock double butterfly are redundant here."""
    drain_inst = self.nc.sync.drain()
    wait_clock.add_sem_waits(
        drain_inst.ins, ScopedClock({None: tick_clock.global_clock})
    )
    self.nc.all_engine_barrier(sem_only=True)
    popped = self.nc._tile_sem_poison_stack.pop()
    assert popped is self._sem_poison
    self.nc.clear_and_free_semaphores(list(self.sems.allocated().values()))
    self.nc.all_engine_barrier(sem_only=True)

F32 = mybir.dt.float32
F32R = mybir.dt.float32r
AF = mybir.ActivationFunctionType
OP = mybir.AluOpType

N = 512          # rows
D = 256          # embedding dim
NCORES = 8
A = N // NCORES  # anchors per core
MARGIN = 5.0
EPS = 1e-16
BIG = 1e30       # pad-pair kill value
BIGM = 128.0     # same-label mask / bias carrier (power of two)

_cache = {}


def _build(Tp: int):
    """Build the (uniform, SPMD) per-core Bass program for Tp pair tiles."""
    tile.TileContext._drain_and_barrier = _cheap_drain_and_barrier
    bass.Bass.all_engine_barrier = _skip_const_barrier
    nc = bacc.Bacc(None, target_bir_lowering=False, num_swdge_queues=2)

    rept_d = nc.declare_dram_parameter("rept", [128, 2, N], F32, isOutput=False)
    repa_d = nc.declare_dram_parameter("repa", [A, D], F32, isOutput=False)
    repat_d = nc.declare_dram_parameter("repat", [128, 2, A], F32, isOutput=False)
    bigm_d = nc.declare_dram_parameter("bigm", [A, N], mybir.dt.float8e4, isOutput=False)
    sel_d = nc.declare_dram_parameter("sel", [A, Tp * 128], mybir.dt.float8e4, isOutput=False)
    pm_d = nc.declare_dram_parameter("pm", [128, 2 * Tp], F32, isOutput=False)
    out_d = nc.declare_dram_parameter("out", [1, 2 * Tp], F32, isOutput=True)

    with tile.TileContext(nc) as tc:
        with (
            tc.tile_pool(name="singles", bufs=1) as sg,
            tc.tile_pool(name="scr", bufs=2) as scr,
            tc.tile_pool(name="xs", bufs=3) as xs,
            tc.tile_pool(name="ppf", bufs=1, space="PSUM") as ppf,
            tc.tile_pool(name="ppg", bufs=4, space="PSUM") as ppg,
            tc.tile_pool(name="ppd", bufs=1, space="PSUM") as ppd,
        ):
            iota_f = sg.tile([128, N], F32)
            nc.gpsimd.iota(
                iota_f[:], [[1, N]], channel_multiplier=0,
                allow_small_or_imprecise_dtypes=True,
            )
            ones = sg.tile([128, 1], F32)
            nc.vector.memset(ones[:], 1.0)
            onesr = sg.tile([128, 1], F32R)
            nc.vector.tensor_copy(onesr[:], ones[:])
            ones1 = sg.tile([1, A], F32)
            nc.vector.memset(ones1[:], 1.0)
            ones1r = sg.tile([1, A], F32R)
            nc.vector.tensor_copy(ones1r[:], ones1[:])
            dmy = sg.tile([1, 1], F32)
            nc.scalar.activation(dmy[:], ones[0:1, :], AF.Sqrt, bias=ones[0:1, :])

            # input loads, spread across the two HWDGE queues; rep first
            # (the row-norm chain below is the longest dependency chain)
            rept_s = sg.tile([128, 2, N], F32)     # rept[p, c, j] = rep[j, c*128+p]
            for q in range(4):
                eng = nc.sync if q % 2 == 0 else nc.scalar
                eng.dma_start(
                    rept_s[:, q // 2, (q % 2) * 256:(q % 2) * 256 + 256],
                    rept_d[:, q // 2, (q % 2) * 256:(q % 2) * 256 + 256],
                )
            repat_s = sg.tile([128, 2, A], F32)    # repat[p, c, a] = repa[a, c*128+p]
            nc.gpsimd.dma_start(repat_s[:], repat_d[:])
            repa_s = sg.tile([A, D], F32)
            nc.gpsimd.dma_start(repa_s[:], repa_d[:])
            bigm_s = sg.tile([A, N], mybir.dt.float8e4)
            nc.gpsimd.dma_start(bigm_s[:], bigm_d[:])
            sel_s = sg.tile([A, Tp * 128], mybir.dt.float8e4)
            nc.gpsimd.dma_start(sel_s[:], sel_d[:])
            pm_s = sg.tile([128, 2 * Tp], F32)     # [:, :Tp] pidx, [:, Tp:] margin
            nc.gpsimd.dma_start(pm_s[:], pm_d[:])

            # float32r operand copies (PE consumes pre-rounded data), per
            # chunk so each overlaps the other chunk's DMA
            reptr = sg.tile([128, 2, N], F32R)
            for c in range(2):
                nc.vector.tensor_copy(reptr[:, c, :], rept_s[:, c, :])
            negTa = sg.tile([128, 2, A], F32R)
            nc.vector.tensor_scalar_mul(negTa[:], repat_s[:], -2.0)

            # d2[a, j] = sq_a + sq_j - 2*dot: start the big -2*dot matmuls as
            # soon as the casts land; the sq_j rank-1 terms join the group last
            d2_p = ppd.tile([A, N], F32, tag="d2")
            nc.tensor.matmul(d2_p[:], negTa[:, 0, :], reptr[:, 0, :],
                             start=True, stop=False, skip_group_check=True)
            nc.tensor.matmul(d2_p[:], negTa[:, 1, :], reptr[:, 1, :],
                             start=False, stop=False, skip_group_check=True)

            # sq_row[1, j] = ||rep_j||^2 = ones.T @ (rept * rept)
            sqsq = sg.tile([128, 2, N], F32R)
            for c in range(2):
                nc.vector.tensor_mul(sqsq[:, c, :], rept_s[:, c, :], rept_s[:, c, :])
            sqrow_p = ppf.tile([1, N], F32, tag="fin")
            nc.tensor.matmul(sqrow_p[:], onesr[:], sqsq[:, 0, :], start=True,
                             stop=False, skip_group_check=True)
            nc.tensor.matmul(sqrow_p[:], onesr[:], sqsq[:, 1, :], start=False,
                             stop=True, skip_group_check=True)
            sqrowr = sg.tile([1, N], F32R)
            nc.vector.tensor_copy(sqrowr[:], sqrow_p[:])
            nc.tensor.matmul(d2_p[:], ones1r[:], sqrowr[:], start=False, stop=True,
                             skip_group_check=True)

            # sq_anch[64,1] = ||rep_a||^2
            sqa_scr = scr.tile([A, D], F32, tag="sqa")
            sqanch = sg.tile([A, 1], F32)
            nc.vector.scalar_tensor_tensor(
                out=sqa_scr[:], in0=repa_s[:], scalar=1.0, in1=repa_s[:],
                op0=OP.mult, op1=OP.mult, accum_out=sqanch[:],
            )

            selr = sg.tile([A, Tp * 128], F32R)
            nc.vector.tensor_copy(selr[:], sel_s[:])

            # ym = sqrt(d2 + 0.25) + BIGM*same: the +0.25 keeps the (masked)
            # diagonal's rounding noise out of sqrt's domain; its effect on
            # d_ap - d_ak cancels to ~5e-4
            sqanchb = xs.tile([A, 1], F32, tag="sqb")
            nc.vector.tensor_scalar(sqanchb[:], sqanch[:], 0.25, None, OP.add)
            dtmp = scr.tile([A, N], F32, tag="dtmp")
            nc.scalar.activation(dtmp[:], d2_p[:], AF.Sqrt, bias=sqanchb[:])
            ym = sg.tile([A, N], F32R)
            nc.vector.tensor_add(ym[:], bigm_s[:], dtmp[:])

            # pair tiles
            SC = sg.tile([128, 2 * Tp], F32)
            nc.vector.memset(SC[:], 0.0)
            relbig = sg.tile([128, Tp, N], F32)
            for t in range(Tp):
                gy = ppg.tile([128, N], F32, tag="gy")
                nc.tensor.matmul(gy[:], selr[:, t * 128:(t + 1) * 128], ym[:],
                                 start=True, stop=True)

                stt = scr.tile([128, N], F32, tag="stt")
                xv = xs.tile([128, 1], F32, tag="xv")
                nc.vector.scalar_tensor_tensor(
                    out=stt[:], in0=iota_f[:], scalar=pm_s[:, t:t + 1], in1=gy[:],
                    op0=OP.is_equal, op1=OP.mult, accum_out=xv[:],
                )
                xp = xs.tile([128, 1], F32, tag="xp")
                nc.vector.tensor_scalar(
                    xp[:], xv[:], pm_s[:, Tp + t:Tp + t + 1], None, OP.add
                )

                nc.scalar.activation(
                    relbig[:, t, :], gy[:], AF.Relu, bias=xp[:], scale=-1.0,
                    accum_out=SC[:, t:t + 1],
                )

            # counts: relu output is positive exactly where a triplet is
            # positive, so two wide scans replace five per-tile ones
            h = (Tp + 1) // 2
            nc.vector.tensor_scalar(
                relbig[:, 0:h, :], relbig[:, 0:h, :], 0.0, 0.0, OP.is_gt, OP.add,
                accum_out=SC[:, Tp:Tp + 1],
            )
            if Tp > h:
                nc.vector.tensor_scalar(
                    relbig[:, h:Tp, :], relbig[:, h:Tp, :], 0.0, 0.0,
                    OP.is_gt, OP.add,
                    accum_out=SC[:, Tp + 1:Tp + 2],
                )

            # partition-sum S and C columns -> [1, 2*Tp]
            fin_p = ppf.tile([1, 2 * Tp], F32, tag="fin")
            nc.tensor.matmul(fin_p[:], ones[:], SC[:], start=True, stop=True)
            outsb = sg.tile([1, 2 * Tp], F32)
            nc.vector.tensor_copy(outsb[:], fin_p[:])
            nc.sync.dma_start(out_d[:], outsb[:])

    nc.finalize()
    return nc


def _prep(rep: np.ndarray, labels: np.ndarray):
    """Host-side integer/mask/layout prep: shard anchors, enumerate pairs."""
    rep = np.ascontiguousarray(np.asarray(rep, dtype=np.float32))
    labels = np.asarray(labels)
    same = labels[:, None] == labels[None, :]

    # rep.T packed [128, 2, N]: rept[p, c, j] = rep[j, c*128 + p]
    rept = np.ascontiguousarray(
        rep.T.reshape(2, 128, N).transpose(1, 0, 2)
    )

    pairs = []
    for c in range(NCORES):
        base = c * A
        prs = [
            (j, p)
            for j in range(A)
            for p in np.nonzero(same[base + j])[0]
            if p != base + j
        ]
        pairs.append(prs)
    Tp = max(1, max((len(p) + 127) // 128 for p in pairs))

    in_maps = []
    for c in range(NCORES):
        base = c * A
        repa = rep[base:base + A]
        repat = np.ascontiguousarray(
            repa.T.reshape(2, 128, A).transpose(1, 0, 2)
        )
        bigm = np.where(same[base:base + A], BIGM, 0.0).astype(ml_dtypes.float8_e4m3)
        sel = np.zeros((A, Tp * 128), ml_dtypes.float8_e4m3)
        pm = np.zeros((128, 2 * Tp), np.float32)
        pm[:, Tp:] = -BIG
        for i, (j, p) in enumerate(pairs[c]):
            t, r = divmod(i, 128)
            sel[j, i] = 1.0
            pm[r, t] = p
            pm[r, Tp + t] = MARGIN - BIGM
        in_maps.append({
            "rept": rept,
            "repa": repa,
            "repat": repat,
            "bigm": bigm,
            "sel": sel,
            "pm": pm,
        })
    return Tp, in_maps


def _run(rep, labels, trace=False):
    Tp, in_maps = _prep(rep, labels)
    if Tp not in _cache:
        _cache[Tp] = _build(Tp)
    nc = _cache[Tp]
    res = run_bass_kernel_spmd(nc, in_maps, list(range(NCORES)), trace=trace)
    outs = np.stack([res.results[c]["out"][0] for c in range(NCORES)])  # [8, 2*Tp]
    S = float(outs[:, :Tp].sum())
    C = float(outs[:, Tp:].sum())
    loss = np.float32(S / (C + EPS))
    return np.asarray(loss, dtype=np.float32), res


def kernel(rep, labels):
    loss, _ = _run(rep, labels, trace=False)
    return loss



# revision 5
# speedup vs baseline: 1.1082x; 1.1082x over previous
"""BatchAllTripletLoss kernel for 8 Trainium2 NeuronCores.

Reference computation:
    pd = pairwise_euclidean(rep)                        # [512, 512]
    tl[a,p,k] = relu(pd[a,p] - pd[a,k] + 5.0) * mask    # [512, 512, 512]
    loss = sum(tl) / (count(tl > eps) + eps)

The mask (p!=a, k!=a, p!=k, label[p]==label[a], label[k]!=label[a])
collapses: valid triplets are exactly (same-label anchor-positive pairs)
x (k with a different label).  With 64 labels over 512 rows there are
only ~4100 (a,p) pairs, so each core processes its anchors' pairs as
rows of [128-pair, 512-k] tiles:

  per core (64 anchors, pair-count balanced across cores):
    sqrow[1,512] = ones.T @ rept^2                  PE column norms
    d2[64,512]   = -2 a.rep' + sq_a + sq_k          PE (+2 rank-1 terms)
    ym[0:64]     = sqrt(d2 + 0.25)                  ACT -> bf16
    ym[64:128]   = labmask (BIGM per label row)     host data
    per pair tile t (128 pairs):
      gy  = [sel_onehot ; label_onehot].T @ ym      PE: d(a,k) + BIGM*same
      xv  = sum_k (iota==pidx)*gy                   DVE: = d(a,p) + BIGM
      xp  = xv + (margin - BIGM)                    DVE
      S_t = sum_k relu(xp - gy)  -> relbig bf16     ACT accum
      C_t = sum_k (relbig > 0)                      DVE accum (bf16 2x)
    out[1,2*Tp]  = ones.T @ [S | C]                 PE partition sum

Everything runs in bf16 (one matmul-input rounding; the final loss is a
mean over ~2M triplets so the mean-zero rounding noise averages out to
~1e-3, far inside the 2e-2 gate).  BIGM = 128 is bf16-exact and both
masks out same-label k columns and carries the bias through the gather.
Columns are permuted per core so its 64 anchors sit at columns 0:63
(sq_anchor is then just sqrow[0:64]); anchors are assigned to cores by
balanced pair-count so Tp = max ceil(pairs/128) is minimal.  All inputs
arrive as two [128, W] bf16 blocks - one dma_start on each HWDGE queue
(pidx columns are fp16 bytes inside the bf16 block, bitcast on device).
The 8 partial (sum, count) pairs are reduced on the host (the
all-reduce of the sharding hint).  Host-side prep is integer/mask/
layout logic only (plus the exact *-2 fold); all float arithmetic runs
on device.
"""

import ml_dtypes
import numpy as np

import concourse.bass as bass
import concourse.tile as tile
from concourse import bacc, mybir
from concourse.bass_utils import run_bass_kernel_spmd
from concourse.vector_clock import ScopedClock


_orig_aeb = bass.Bass.all_engine_barrier


def _skip_const_barrier(self, *, sem_only=False):
    if not getattr(self, "_aeb_skipped_once", False):
        self._aeb_skipped_once = True
        return
    return _orig_aeb(self, sem_only=sem_only)


def _cheap_drain_and_barrier(self, tick_clock, wait_clock):
    """Exit protocol with sequencer-only barriers: the SP drain already
    waits out every engine/DMA tick of the tile clock, so the per-engine
    pipeline drains of the stock double butterfly are redundant here."""
    drain_inst = self.nc.sync.drain()
    wait_clock.add_sem_waits(
        drain_inst.ins, ScopedClock({None: tick_clock.global_clock})
    )
    self.nc.all_engine_barrier(sem_only=True)
    popped = self.nc._tile_sem_poison_stack.pop()
    assert popped is self._sem_poison
    self.nc.clear_and_free_semaphores(list(self.sems.allocated().values()))
    self.nc.all_engine_barrier(sem_only=True)

F32 = mybir.dt.float32
BF16 = mybir.dt.bfloat16
F16 = mybir.dt.float16
AF = mybir.ActivationFunctionType
OP = mybir.AluOpType

N = 512          # rows
D = 256          # embedding dim
NCORES = 8
A = N // NCORES  # anchors per core
NL = 64          # label count
MARGIN = 5.0
EPS = 1e-16
BIGM = 128.0     # same-label mask / bias carrier (bf16-exact power of two)

_cache = {}


def _build(Tp: int):
    """Build the (uniform, SPMD) per-core Bass program for Tp pair tiles."""
    tile.TileContext._drain_and_barrier = _cheap_drain_and_barrier
    bass.Bass.all_engine_barrier = _skip_const_barrier
    nc = bacc.Bacc(None, target_bir_lowering=False, num_swdge_queues=1)
    # the gpsimd software-DGE queue is unused (both input blocks and the
    # output ride the two HWDGE queues); shrink it to one ring so the
    # runtime's per-ring boot/teardown semaphore protocol stays short.
    for q in nc.m.queues:
        if q.engine == mybir.EngineType.Pool:
            q.num_queues = 1

    # BLK_B column layout (bf16):
    #   [0:128)           repat2[p, c, a] = -2*rep[perm[a], c*128+p]
    #   [128:640)         ym region: rows 64:128 labmask, rows 0:64 junk
    #   [640:640+128*Tp)  sel: anchor one-hot + label one-hot per pair
    #   [.. +Tp)          pidx per tile (fp16 bytes)
    YM0 = 128
    SEL0 = 640
    PM0 = SEL0 + 128 * Tp
    WB = PM0 + Tp

    blka_d = nc.declare_dram_parameter("blka", [128, 2, N], BF16, isOutput=False)
    blkb_d = nc.declare_dram_parameter("blkb", [128, WB], BF16, isOutput=False)
    out_d = nc.declare_dram_parameter("out", [1, 2 * Tp], F32, isOutput=True)

    with tile.TileContext(nc) as tc:
        with (
            tc.tile_pool(name="singles", bufs=1) as sg,
            tc.tile_pool(name="scr", bufs=2) as scr,
            tc.tile_pool(name="xs", bufs=3) as xs,
            tc.tile_pool(name="rb", bufs=2) as rb,
            tc.tile_pool(name="ppf", bufs=1, space="PSUM") as ppf,
            tc.tile_pool(name="ppg", bufs=4, space="PSUM") as ppg,
            tc.tile_pool(name="ppd", bufs=1, space="PSUM") as ppd,
        ):
            # input loads, one per HWDGE queue, ahead of everything else
            blka_s = sg.tile([128, 2, N], BF16)
            blkb_s = sg.tile([128, WB], BF16)
            with tc.high_priority():
                nc.sync.dma_start(blka_s[:], blka_d[:])
                nc.scalar.dma_start(blkb_s[:], blkb_d[:])

            iota_f = sg.tile([128, N], F32)
            nc.gpsimd.iota(
                iota_f[:], [[1, N]], channel_multiplier=0,
                allow_small_or_imprecise_dtypes=True,
            )
            ones = sg.tile([128, 1], F32)
            nc.vector.memset(ones[:], 1.0)
            onesb = sg.tile([128, 1], BF16)
            nc.vector.memset(onesb[:], 1.0)
            ones1 = sg.tile([1, N], BF16)   # rank-1 rhs / lhsT rows of ones
            nc.vector.memset(ones1[:], 1.0)
            g4 = sg.tile([A, 1], F32)       # sqrt-domain guard bias
            nc.vector.memset(g4[:], 4.0)
            # dummy activations pull the ACT table loads to program start
            dmy = sg.tile([1, 1], F32)
            nc.scalar.activation(dmy[:], ones[0:1, :], AF.Sqrt, bias=ones[0:1, :])
            nc.scalar.activation(dmy[:], ones[0:1, :], AF.Relu, bias=ones[0:1, :])

            ymfull = blkb_s[:, YM0:YM0 + N]          # rows 64:128 = labmask
            pm16 = blkb_s[:, PM0:PM0 + Tp].bitcast(F16)

            # sqrow[1, j] = ||rep_j||^2 = ones.T @ (rept * rept)
            sqsq = scr.tile([128, 2, N], BF16, tag="sqsq")
            for c in range(2):
                nc.vector.tensor_mul(sqsq[:, c, :], blka_s[:, c, :], blka_s[:, c, :])
            sqrow_p = ppf.tile([1, N], F32, tag="fin")
            nc.tensor.matmul(sqrow_p[:], onesb[:], sqsq[:, 0, :], start=True,
                             stop=False, skip_group_check=True)
            nc.tensor.matmul(sqrow_p[:], onesb[:], sqsq[:, 1, :], start=False,
                             stop=True, skip_group_check=True)
            sqrow = sg.tile([1, N], BF16)
            nc.vector.tensor_copy(sqrow[:], sqrow_p[:])

            # d2[a, j] = -2*dot + sq_k (rank-1) + sq_a (rank-1)
            d2_p = ppd.tile([A, N], F32, tag="d2")
            nc.tensor.matmul(d2_p[:], blkb_s[:, 0:A], blka_s[:, 0, :],
                             start=True, stop=False, skip_group_check=True)
            nc.tensor.matmul(d2_p[:], blkb_s[:, A:2 * A], blka_s[:, 1, :],
                             start=False, stop=False, skip_group_check=True)
            nc.tensor.matmul(d2_p[:], ones1[:, 0:A], sqrow[:], start=False,
                             stop=False, skip_group_check=True)
            nc.tensor.matmul(d2_p[:], sqrow[:, 0:A], ones1[:], start=False,
                             stop=True, skip_group_check=True)

            # ym rows 0:64 = sqrt(d2 + 4): the +4 keeps the (masked)
            # diagonal's bf16 rounding noise (observed +-2) out of sqrt's
            # domain; the shift cancels in d_ap - d_ak to ~3e-5 (measured)
            nc.scalar.activation(ymfull[0:A, :], d2_p[:], AF.Sqrt, bias=g4[:])

            # pair tiles
            SC = sg.tile([128, 2 * Tp], F32)
            nc.vector.memset(SC[:], 0.0)
            for t in range(Tp):
                gy = ppg.tile([128, N], F32, tag="gy")
                nc.tensor.matmul(gy[:], blkb_s[:, SEL0 + t * 128:SEL0 + (t + 1) * 128],
                                 ymfull, start=True, stop=True)

                stt = scr.tile([128, N], BF16, tag="stt")
                xv = xs.tile([128, 1], F32, tag="xv")
                nc.vector.scalar_tensor_tensor(
                    out=stt[:], in0=iota_f[:], scalar=pm16[:, t:t + 1], in1=gy[:],
                    op0=OP.is_equal, op1=OP.mult, accum_out=xv[:],
                )
                xp = xs.tile([128, 1], F32, tag="xp")
                nc.vector.tensor_scalar(xp[:], xv[:], MARGIN - BIGM, None, OP.add)

                relbig = rb.tile([128, N], BF16, tag="relbig")
                nc.scalar.activation(
                    relbig[:], gy[:], AF.Relu, bias=xp[:], scale=-1.0,
                    accum_out=SC[:, t:t + 1],
                )
                junk = rb.tile([128, N], BF16, tag="junk")
                nc.vector.tensor_scalar(
                    junk[:], relbig[:], 0.0, 0.0, OP.is_gt, OP.add,
                    accum_out=SC[:, Tp + t:Tp + t + 1],
                )

            # partition-sum S and C columns -> [1, 2*Tp]
            fin_p = ppf.tile([1, 2 * Tp], F32, tag="fin")
            nc.tensor.matmul(fin_p[:], ones[:], SC[:], start=True, stop=True)
            outsb = sg.tile([1, 2 * Tp], F32)
            nc.vector.tensor_copy(outsb[:], fin_p[:])
            nc.sync.dma_start(out_d[:], outsb[:])

    nc.finalize()
    return nc


def _prep(rep: np.ndarray, labels: np.ndarray):
    """Host-side integer/mask/layout prep: balance anchors, enumerate pairs."""
    rep = np.asarray(rep, dtype=np.float32)
    labels = np.asarray(labels).astype(np.int64)
    repb = rep.astype(ml_dtypes.bfloat16)
    repb2 = (-2.0 * rep).astype(ml_dtypes.bfloat16)

    members = {l: np.nonzero(labels == l)[0] for l in range(NL)}
    npairs = np.array([len(members[labels[a]]) - 1 for a in range(N)])

    # balanced partition: 8 bins of exactly 64 anchors, minimizing max
    # total pair count (greedy LPT under the exact-size constraint)
    order = np.argsort(-npairs, kind="stable")
    bins = [[] for _ in range(NCORES)]
    loads = [0] * NCORES
    for a in order:
        cands = [c for c in range(NCORES) if len(bins[c]) < A]
        c = min(cands, key=lambda c: (loads[c], len(bins[c])))
        bins[c].append(int(a))
        loads[c] += int(npairs[a])
    Tp = max(1, (max(loads) + 127) // 128)

    in_maps = []
    YM0, SEL0 = 128, 640
    PM0 = SEL0 + 128 * Tp
    WB = PM0 + Tp
    for c in range(NCORES):
        anchors = bins[c]
        rest = [j for j in range(N) if j not in set(anchors)]
        perm = np.array(anchors + rest)
        col_of = np.empty(N, np.int64)
        col_of[perm] = np.arange(N)

        # blka[p, ch, col] = rep[perm[col], ch*128+p]
        blka = np.ascontiguousarray(
            repb[perm].T.reshape(2, 128, N).transpose(1, 0, 2)
        )
        blkb = np.zeros((128, WB), ml_dtypes.bfloat16)
        # repat2[p, ch*64+a] = -2*rep[perm[a], ch*128+p]
        blkb[:, 0:128] = repb2[perm[:A]].T.reshape(2, 128, A).transpose(1, 0, 2)\
            .reshape(128, 2 * A)
        # labmask rows 64:128 of the ym region
        lab_cols = labels[perm]                       # label of column k
        lm = (lab_cols[None, :] == np.arange(NL)[:, None])
        blkb[A:128, YM0:YM0 + N] = np.where(lm, BIGM, 0.0)
        # sel one-hots + pidx (fp16 bytes inside the bf16 block)
        pm = np.zeros((128, Tp), np.float16)
        i = 0
        for j, a in enumerate(anchors):
            la = int(labels[a])
            for p in members[la]:
                if p == a:
                    continue
                t, r = divmod(i, 128)
                blkb[j, SEL0 + i] = 1.0
                blkb[A + la, SEL0 + i] = 1.0
                pm[r, t] = np.float16(col_of[p])
                i += 1
        blkb[:, PM0:PM0 + Tp] = pm.view(ml_dtypes.bfloat16)
        in_maps.append({"blka": blka, "blkb": blkb})
    return Tp, in_maps


def _run(rep, labels, trace=False):
    Tp, in_maps = _prep(rep, labels)
    if Tp not in _cache:
        _cache[Tp] = _build(Tp)
    nc = _cache[Tp]
    res = run_bass_kernel_spmd(nc, in_maps, list(range(NCORES)), trace=trace)
    outs = np.stack([res.results[c]["out"][0] for c in range(NCORES)])  # [8, 2*Tp]
    S = float(outs[:, :Tp].sum())
    C = float(outs[:, Tp:].sum())
    loss = np.float32(S / (C + EPS))
    return np.asarray(loss, dtype=np.float32), res


def kernel(rep, labels):
    loss, _ = _run(rep, labels, trace=False)
    return loss


# revision 6
# speedup vs baseline: 1.2347x; 1.1142x over previous
"""BatchAllTripletLoss kernel for 8 Trainium2 NeuronCores.

Reference computation:
    pd = pairwise_euclidean(rep)                        # [512, 512]
    tl[a,p,k] = relu(pd[a,p] - pd[a,k] + 5.0) * mask    # [512, 512, 512]
    loss = sum(tl) / (count(tl > eps) + eps)

The mask (p!=a, k!=a, p!=k, label[p]==label[a], label[k]!=label[a])
collapses: valid triplets are exactly (same-label anchor-positive pairs)
x (k with a different label).  With 64 labels over 512 rows there are
only ~4100 (a,p) pairs, so each core processes its anchors' pairs as
rows of [128-pair, 512-k] tiles:

  per core (64 anchors, pair-count balanced across cores):
    sqrow[1,512] = ones.T @ rept^2                  PE column norms
    sqanch[64,1] = rowsum(repa^2)                   DVE
    d2[64,512]   = -2 a.rep' + sq_k (rank-1)        PE
    ym[0:64]     = sqrt(d2 + sqanch + 4)            ACT -> bf16
    ym[64:128]   = labmask (BIGM per label row)     host data
    per pair tile t (128 pairs):
      gy  = [sel_onehot ; label_onehot].T @ ym      PE: d(a,k) + BIGM*same
      xv  = sum_k (iota==pidx)*gy                   DVE: = d(a,p) + BIGM
      xp  = xv + (margin - BIGM)                    GPSIMD
      S_t = sum_k relu(xp - gy)  -> relbig bf16     ACT accum
      ind = relbig > 0  (bf16, 4x)                  DVE
      cnt[1,512] += ones.T @ ind                    PE (accumulate)
    C = sum(cnt) + last tile direct-counted         DVE
  host sums the 8 cores' partial S rows / C cells (the all-reduce).

Everything runs in bf16 (one matmul-input rounding; the final loss is a
mean over ~2M triplets so the mean-zero rounding noise averages out to
~1e-4, far inside the 2e-2 gate).  BIGM = 128 is bf16-exact and both
masks out same-label k columns and carries the bias through the gather
(the label one-hot block makes the gather contraction exactly 128).
Columns are permuted per core so its 64 anchors sit at columns 0:63;
anchors are assigned to cores by balanced pair-count so Tp is minimal.
All inputs arrive as two [128, W] bf16 blocks - one dma_start on each
HWDGE queue, ordered so the critical block rides the early-booting
Activation queue (pidx columns are fp16 bytes, bitcast on device).  A
burst of throwaway matmuls during the DMA wait warms the PE HAM clock
gate to 2.4 GHz before the real matmuls.  Host-side prep is integer/
mask/layout logic only (plus the exact *-2 fold); all float arithmetic
runs on device.
"""

import ml_dtypes
import numpy as np

import concourse.bass as bass
import concourse.tile as tile
from concourse import bacc, mybir
from concourse.bass_utils import run_bass_kernel_spmd
from concourse.vector_clock import ScopedClock


_orig_aeb = bass.Bass.all_engine_barrier


def _skip_const_barrier(self, *, sem_only=False):
    if not getattr(self, "_aeb_skipped_once", False):
        self._aeb_skipped_once = True
        return
    return _orig_aeb(self, sem_only=sem_only)


def _cheap_drain_and_barrier(self, tick_clock, wait_clock):
    """Exit protocol with sequencer-only barriers: the SP drain already
    waits out every engine/DMA tick of the tile clock, so the per-engine
    pipeline drains of the stock double butterfly are redundant here."""
    drain_inst = self.nc.sync.drain()
    wait_clock.add_sem_waits(
        drain_inst.ins, ScopedClock({None: tick_clock.global_clock})
    )
    self.nc.all_engine_barrier(sem_only=True)
    popped = self.nc._tile_sem_poison_stack.pop()
    assert popped is self._sem_poison
    self.nc.clear_and_free_semaphores(list(self.sems.allocated().values()))
    self.nc.all_engine_barrier(sem_only=True)

F32 = mybir.dt.float32
BF16 = mybir.dt.bfloat16
F16 = mybir.dt.float16
AF = mybir.ActivationFunctionType
OP = mybir.AluOpType

N = 512          # rows
D = 256          # embedding dim
NCORES = 8
A = N // NCORES  # anchors per core
NL = 64          # label count
MARGIN = 5.0
EPS = 1e-16
BIGM = 128.0     # same-label mask / bias carrier (bf16-exact power of two)
NWARM = 5        # PE warmup matmuls

_cache = {}


def _build(Tp: int):
    """Build the (uniform, SPMD) per-core Bass program for Tp pair tiles."""
    tile.TileContext._drain_and_barrier = _cheap_drain_and_barrier
    bass.Bass.all_engine_barrier = _skip_const_barrier
    nc = bacc.Bacc(None, target_bir_lowering=False, num_swdge_queues=1)
    # the gpsimd software-DGE queue is unused (both input blocks and the
    # output ride the two HWDGE queues); shrink it to one ring so the
    # runtime's per-ring setup protocol stays short.
    for q in nc.m.queues:
        if q.engine == mybir.EngineType.Pool:
            q.num_queues = 1

    # blka (critical, on the Activation HWDGE queue):
    #   [0:1024)          rept[p, c*512+j] = rep[perm[j], c*128+p]
    #   [1024:1152)       repat2[p, c*64+a] = -2*rep[perm[a], c*128+p]
    # blkb (on the SP HWDGE queue):
    #   [0:512)           ym region: rows 64:128 labmask, rows 0:64 junk
    #   [512:512+128*Tp)  sel: anchor one-hot + label one-hot per pair
    #   [.. +Tp)          pidx per tile (fp16 bytes)
    #   [.. +256)         repa[a, d] = rep[perm[a], d] (rows 0:64)
    YM0 = 0
    SEL0 = 512
    PM0 = SEL0 + 128 * Tp
    RA0 = PM0 + Tp
    WB = RA0 + D

    blka_d = nc.declare_dram_parameter("blka", [128, 1152], BF16, isOutput=False)
    blkb_d = nc.declare_dram_parameter("blkb", [128, WB], BF16, isOutput=False)
    out_d = nc.declare_dram_parameter("out", [128, Tp + 2], F32, isOutput=True)

    with tile.TileContext(nc) as tc:
        with (
            tc.tile_pool(name="singles", bufs=1) as sg,
            tc.tile_pool(name="scr", bufs=2) as scr,
            tc.tile_pool(name="xs", bufs=3) as xs,
            tc.tile_pool(name="rb", bufs=2) as rb,
            tc.tile_pool(name="ppf", bufs=1, space="PSUM") as ppf,
            tc.tile_pool(name="ppg", bufs=4, space="PSUM") as ppg,
            tc.tile_pool(name="ppd", bufs=1, space="PSUM") as ppd,
            tc.tile_pool(name="ppc", bufs=1, space="PSUM") as ppc,
        ):
            # input loads, one per HWDGE queue, ahead of everything else
            blka_s = sg.tile([128, 1152], BF16)
            blkb_s = sg.tile([128, WB], BF16)
            with tc.high_priority():
                nc.scalar.dma_start(blka_s[:], blka_d[:])
                nc.sync.dma_start(blkb_s[:], blkb_d[:])

            # PE warmup: throwaway 512-wide matmuls on a memset operand keep
            # the PE busy through the HAM activity window while the inputs
            # stream in, so the real matmuls run at 2.4 GHz instead of 1.2
            zs = sg.tile([128, N], BF16)
            nc.vector.memset(zs[:], 1.0)
            onesb = sg.tile([128, 1], BF16)
            nc.vector.memset(onesb[:], 1.0)
            cnt_p = ppc.tile([1, N], F32, tag="cnt")
            for _ in range(NWARM):
                nc.tensor.matmul(cnt_p[:], onesb[:], zs[:], start=True, stop=True)

            iota_f = sg.tile([128, N], F32)
            nc.gpsimd.iota(
                iota_f[:], [[1, N]], channel_multiplier=0,
                allow_small_or_imprecise_dtypes=True,
            )
            ones1 = sg.tile([1, N], BF16)   # rank-1 lhsT/ones row
            nc.vector.memset(ones1[:], 1.0)
            negc = sg.tile([128, 1], F32)   # margin - BIGM for the xp hop
            nc.vector.memset(negc[:], MARGIN - BIGM)
            # dummy activations pull the ACT table load to program start
            dmy = sg.tile([1, 1], F32)
            nc.scalar.activation(dmy[:], negc[0:1, :], AF.Sqrt, bias=negc[0:1, :])
            nc.scalar.activation(dmy[:], negc[0:1, :], AF.Relu, bias=negc[0:1, :])

            ymfull = blkb_s[:, YM0:YM0 + N]          # rows 64:128 = labmask
            pm16 = blkb_s[:, PM0:PM0 + Tp].bitcast(F16)
            repa = blkb_s[0:A, RA0:RA0 + D]

            # sqrow[1, j] = ||rep_j||^2 = ones.T @ (rept * rept)
            sqsq = scr.tile([128, 2, N], BF16, tag="sqsq")
            for c in range(2):
                nc.vector.tensor_mul(
                    sqsq[:, c, :], blka_s[:, c * N:(c + 1) * N],
                    blka_s[:, c * N:(c + 1) * N],
                )
            sqrow_p = ppf.tile([1, N], F32, tag="sqr")
            nc.tensor.matmul(sqrow_p[:], onesb[:], sqsq[:, 0, :], start=True,
                             stop=False, skip_group_check=True)
            nc.tensor.matmul(sqrow_p[:], onesb[:], sqsq[:, 1, :], start=False,
                             stop=True, skip_group_check=True)
            sqrow = sg.tile([1, N], BF16)
            nc.vector.tensor_copy(sqrow[:], sqrow_p[:])

            # sqanch[a] = ||rep_a||^2 + 4: the +4 keeps the (masked)
            # diagonal's bf16 rounding noise (observed +-2) out of sqrt's
            # domain; the shift cancels in d_ap - d_ak to ~3e-5 (measured)
            sqa_scr = scr.tile([A, D], BF16, tag="sqa")
            sqanch = xs.tile([A, 1], F32, tag="sqv")
            nc.vector.scalar_tensor_tensor(
                out=sqa_scr[:], in0=repa, scalar=1.0, in1=repa,
                op0=OP.mult, op1=OP.mult, accum_out=sqanch[:],
            )
            sqanchb = xs.tile([A, 1], F32, tag="sqb")
            nc.vector.tensor_scalar(sqanchb[:], sqanch[:], 4.0, None, OP.add)

            # d2[a, j] = -2*dot + sq_k (rank-1); sq_a rides the sqrt bias
            d2_p = ppd.tile([A, N], F32, tag="d2")
            nc.tensor.matmul(d2_p[:], blka_s[:, 1024:1024 + A], blka_s[:, 0:N],
                             start=True, stop=False, skip_group_check=True)
            nc.tensor.matmul(d2_p[:], blka_s[:, 1024 + A:1152], blka_s[:, N:2 * N],
                             start=False, stop=False, skip_group_check=True)
            nc.tensor.matmul(d2_p[:], ones1[:, 0:A], sqrow[:], start=False,
                             stop=True, skip_group_check=True)

            nc.scalar.activation(ymfull[0:A, :], d2_p[:], AF.Sqrt, bias=sqanchb[:])

            # pair tiles
            SC = sg.tile([128, Tp + 2], F32)
            nc.vector.memset(SC[:], 0.0)
            for t in range(Tp):
                gy = ppg.tile([128, N], F32, tag="gy")
                nc.tensor.matmul(gy[:], blkb_s[:, SEL0 + t * 128:SEL0 + (t + 1) * 128],
                                 ymfull, start=True, stop=True)

                stt = scr.tile([128, N], BF16, tag="stt")
                xv = xs.tile([128, 1], F32, tag="xv")
                nc.vector.scalar_tensor_tensor(
                    out=stt[:], in0=iota_f[:], scalar=pm16[:, t:t + 1], in1=gy[:],
                    op0=OP.is_equal, op1=OP.mult, accum_out=xv[:],
                )
                xp = xs.tile([128, 1], F32, tag="xp")
                nc.gpsimd.tensor_add(xp[:], xv[:], negc[:])

                relbig = rb.tile([128, N], BF16, tag="relbig")
                nc.scalar.activation(
                    relbig[:], gy[:], AF.Relu, bias=xp[:], scale=-1.0,
                    accum_out=SC[:, t:t + 1],
                )
                if t < Tp - 1:
                    # counts via PE: 4x indicator scan, then a column-sum
                    # matmul accumulating into cnt_p
                    ind = rb.tile([128, N], BF16, tag="ind")
                    nc.vector.tensor_scalar(ind[:], relbig[:], 0.0, None, OP.is_gt)
                    nc.tensor.matmul(cnt_p[:], onesb[:], ind[:],
                                     start=(t == 0), stop=(t == Tp - 2))
                else:
                    # last tile counts directly so the cnt_p reduce below
                    # can overlap it
                    junk = rb.tile([128, N], BF16, tag="ind")
                    nc.vector.tensor_scalar(
                        junk[:], relbig[:], 0.0, 0.0, OP.is_gt, OP.add,
                        accum_out=SC[:, Tp + 1:Tp + 2],
                    )

            # C for tiles 0..Tp-2: reduce the accumulated count columns
            cjunk = scr.tile([1, N], F32, tag="cj")
            nc.vector.tensor_scalar(
                cjunk[:], cnt_p[:], 0.0, 0.0, OP.add, OP.add,
                accum_out=SC[0:1, Tp:Tp + 1],
            )
            nc.sync.dma_start(out_d[:], SC[:])

    nc.finalize()
    return nc


def _prep(rep: np.ndarray, labels: np.ndarray):
    """Host-side integer/mask/layout prep: balance anchors, enumerate pairs."""
    rep = np.asarray(rep, dtype=np.float32)
    labels = np.asarray(labels).astype(np.int64)
    repb = rep.astype(ml_dtypes.bfloat16)
    repb2 = (-2.0 * rep).astype(ml_dtypes.bfloat16)

    members = {l: np.nonzero(labels == l)[0] for l in range(NL)}
    npairs = np.array([len(members[labels[a]]) - 1 for a in range(N)])

    # balanced partition: 8 bins of exactly 64 anchors, minimizing max
    # total pair count (greedy LPT under the exact-size constraint)
    order = np.argsort(-npairs, kind="stable")
    bins = [[] for _ in range(NCORES)]
    loads = [0] * NCORES
    for a in order:
        cands = [c for c in range(NCORES) if len(bins[c]) < A]
        c = min(cands, key=lambda c: (loads[c], len(bins[c])))
        bins[c].append(int(a))
        loads[c] += int(npairs[a])
    Tp = max(2, (max(loads) + 127) // 128)

    in_maps = []
    SEL0 = 512
    PM0 = SEL0 + 128 * Tp
    RA0 = PM0 + Tp
    WB = RA0 + D
    for c in range(NCORES):
        anchors = bins[c]
        rest = [j for j in range(N) if j not in set(anchors)]
        perm = np.array(anchors + rest)
        col_of = np.empty(N, np.int64)
        col_of[perm] = np.arange(N)

        blka = np.zeros((128, 1152), ml_dtypes.bfloat16)
        # rept[p, c*512+j] = rep[perm[j], c*128+p]
        blka[:, 0:1024] = repb[perm].T.reshape(2, 128, N).transpose(1, 0, 2)\
            .reshape(128, 1024)
        # repat2[p, c*64+a] = -2*rep[perm[a], c*128+p]
        blka[:, 1024:1152] = repb2[perm[:A]].T.reshape(2, 128, A)\
            .transpose(1, 0, 2).reshape(128, 2 * A)

        blkb = np.zeros((128, WB), ml_dtypes.bfloat16)
        # labmask rows 64:128 of the ym region
        lab_cols = labels[perm]                       # label of column k
        lm = (lab_cols[None, :] == np.arange(NL)[:, None])
        blkb[A:128, 0:N] = np.where(lm, BIGM, 0.0)
        # sel one-hots + pidx (fp16 bytes inside the bf16 block)
        pm = np.zeros((128, Tp), np.float16)
        i = 0
        for j, a in enumerate(anchors):
            la = int(labels[a])
            for p in members[la]:
                if p == a:
                    continue
                t, r = divmod(i, 128)
                blkb[j, SEL0 + i] = 1.0
                blkb[A + la, SEL0 + i] = 1.0
                pm[r, t] = np.float16(col_of[p])
                i += 1
        blkb[:, PM0:PM0 + Tp] = pm.view(ml_dtypes.bfloat16)
        blkb[0:A, RA0:RA0 + D] = repb[perm[:A]]
        in_maps.append({"blka": blka, "blkb": blkb})
    return Tp, in_maps


def _run(rep, labels, trace=False):
    Tp, in_maps = _prep(rep, labels)
    if Tp not in _cache:
        _cache[Tp] = _build(Tp)
    nc = _cache[Tp]
    res = run_bass_kernel_spmd(nc, in_maps, list(range(NCORES)), trace=trace)
    outs = np.stack([res.results[c]["out"] for c in range(NCORES)])  # [8,128,Tp+2]
    S = float(outs[:, :, 0:Tp].sum())
    C = float(outs[:, 0, Tp].sum()) + float(outs[:, :, Tp + 1].sum())
    loss = np.float32(S / (C + EPS))
    return np.asarray(loss, dtype=np.float32), res


def kernel(rep, labels):
    loss, _ = _run(rep, labels, trace=False)
    return loss
